# revision 53
# baseline (speedup 1.0000x reference)
"""Sliding-window GQA attention (softcap + clipped softmax) on 8 TRN2 NeuronCores.

v3: software-pipelined emission + dtype rebalance.
  - qk matmul in fp16 (1 cyc/col, half the DMA/ldweights of f32r)
  - scores PSUM f32 -> ACT tanh -> t (f32, SBUF)
  - gpsimd adds -1e4 masks on the <=2 boundary blocks of t
  - ACT exp (scale=CAP bias=-CAP) -> E bf16 (range safe: e^-60 needs
    f32-like exponent; fp16 underflows and NaNs the row) + accum l
  - DVE: r = 1/l; y = E*r - 0.03/1.06 (all-SBUF 2-byte -> 4x DVE mode)
  - PE transpose y per 128-block into PSUM quads (fp16, 1 cyc/col)
  - DVE clamp [0, 1/1.06] on quad PSUM->SBUF copy (2x mode)
  - AV in fp16 (V pre-scaled by 1.06 host-side), po PSUM f32
  - output DMA directly from PSUM (no staging copy)
  - 5-stage software pipeline: at step j the engines see
    QK(j) | tanh+mask(j-1) | exp+recip+y(j-2) | transpose+clamp(j-3) |
    AV+outDMA(j-4), so every instruction's deps are satisfied at step
    start and no engine stalls mid-chain.

Sharding: core c -> batch c//4, GQA group c%4 (4 q-heads sharing 1 kv head).
"""

import sys

sys.path.insert(0, "/opt/trn_rl_repo")

import numpy as np

B = 2
S = 2048
HQ = 16
HKV = 4
D = 128
NB = S // 128
WB = 8
CAP = 30.0
SCALE = float(1.0 / np.float32(np.sqrt(np.float32(D))))
MASK_VAL = -1.0e4
NH = 4  # heads per core
M_ITERS = NH * NB  # 64 pipelined iterations per core

_CACHED = {}


def _chunks(wc):
    # PSUM-bank-aligned <=512 chunks (each matmul output within one bank)
    out = []
    while wc > 0:
        c = min(512, wc)
        out.append(c)
        wc -= c
    return out


def _build_bass():
    import concourse.mybir as mybir
    import concourse.tile as tile
    from concourse import bacc
    from contextlib import ExitStack


    f32 = mybir.dt.float32
    f16 = mybir.dt.float16
    bf16 = mybir.dt.bfloat16
    AF = mybir.ActivationFunctionType
    OP = mybir.AluOpType

    nc = bacc.Bacc("TRN2", target_bir_lowering=False, debug=False, num_devices=8)

    qT = nc.dram_tensor("qT", [NH, 128, S], f16, kind="ExternalInput").ap()
    kT = nc.dram_tensor("kT", [128, S], f16, kind="ExternalInput").ap()
    vh = nc.dram_tensor("vh", [S, 128], f16, kind="ExternalInput").ap()
    msk = nc.dram_tensor("msk", [2, 128, 128], f32, kind="ExternalInput").ap()
    idn = nc.dram_tensor("idn", [128, 128], f16, kind="ExternalInput").ap()
    out = nc.dram_tensor("out", [S, NH, 128], f16, kind="ExternalOutput").ap()

    _ORDER = []
    for _h in range(NH):
        _r = range(NB) if _h < NH - 1 else range(NB - 1, -1, -1)
        _ORDER.extend((_h, _i) for _i in _r)

    def it(m):
        return _ORDER[m]  # (head, row-block)

    with tile.TileContext(nc) as tc:
        with ExitStack() as ctx:
            singles = ctx.enter_context(tc.tile_pool(name="singles", bufs=1))
            tpool = ctx.enter_context(tc.tile_pool(name="tpool", bufs=3))
            epool = ctx.enter_context(tc.tile_pool(name="epool", bufs=3))
            ypool = ctx.enter_context(tc.tile_pool(name="ypool", bufs=3))
            apool = ctx.enter_context(tc.tile_pool(name="apool", bufs=3))
            spool = ctx.enter_context(tc.tile_pool(name="spool", bufs=8))
            opool = ctx.enter_context(tc.tile_pool(name="opool", bufs=3))
            psco = ctx.enter_context(tc.tile_pool(name="psco", bufs=2, space="PSUM"))
            ptp = ctx.enter_context(tc.tile_pool(name="ptp", bufs=1, space="PSUM"))
            pout = ctx.enter_context(tc.tile_pool(name="pout", bufs=1, space="PSUM"))

            kT_sb = singles.tile([128, S], f16)
            qa_sb = singles.tile([128, NH, S], f16)
            m_sb = singles.tile([128, 2, 128], f32)
            i_sb = singles.tile([128, 128], f16)
            v_sb = singles.tile([128, NB, 128], f16)
            bcap = singles.tile([128, 1], f32)
            # first QK needs kT[:512] + qa[h0]; masks need m_sb at step 1;
            # v not until step 4 -> load it last
            nc.sync.dma_start(kT_sb[:, :128], kT[:, :128])
            nc.sync.dma_start(qa_sb[:, 0, :], qT[0])
            nc.sync.dma_start(kT_sb[:, 128:512], kT[:, 128:512])
            nc.sync.dma_start(m_sb, msk.rearrange("t p c -> p t c"))
            nc.sync.dma_start(i_sb, idn)
            nc.sync.dma_start(kT_sb[:, 512:], kT[:, 512:])
            for h in range(1, NH):
                nc.sync.dma_start(qa_sb[:, h, :], qT[h])
            for jb in range(NB):
                nc.sync.dma_start(v_sb[:, jb, :], vh[jb * 128:(jb + 1) * 128, :])
            nc.gpsimd.memset(bcap, -CAP)
            # dummy tiny activation: pulls the lazy ACT_TABLE_LOAD to t=0
            # instead of serializing it before the first real tanh
            warm = singles.tile([128, 1], f32)
            nc.scalar.activation(warm, bcap, AF.Tanh)

            state = {}
            for j in range(M_ITERS + 4):
                # ---- stage A: QK scores for iter j -> PSUM ----
                if j < M_ITERS:
                    h, i = it(j)
                    j0 = max(0, i - WB)
                    nW = i - j0 + 1
                    wc = nW * 128
                    ps_full = psco.tile([128, 1152], f32, tag="s")
                    ps = ps_full[:, :wc]
                    c0 = 0
                    for cw in _chunks(wc):
                        mm = nc.tensor.matmul(
                            ps[:, c0:c0 + cw],
                            lhsT=qa_sb[:, h, i * 128:(i + 1) * 128],
                            rhs=kT_sb[:, j0 * 128 + c0: j0 * 128 + c0 + cw],
                            start=True,
                            stop=True,
                        )
                        mm.ins.bass_priority = max(0, tc.cur_priority - 200)
                        c0 += cw
                    state[j] = dict(ps=ps, j0=j0, nW=nW, wc=wc)

                # ---- stage B: tanh + boundary masks for iter j-1 ----
                m = j - 1
                if 0 <= m < M_ITERS:
                    st = state[m]
                    nW = st["nW"]
                    t_full = tpool.tile([128, 1152], f32, tag="t")
                    t = t_full[:, :st["wc"]]
                    nc.scalar.activation(t, st["ps"], AF.Tanh, scale=SCALE)
                    nc.gpsimd.tensor_tensor(
                        t[:, (nW - 1) * 128: nW * 128],
                        t[:, (nW - 1) * 128: nW * 128],
                        m_sb[:, 0, :],
                        op=OP.add,
                    )
                    if it(m)[1] >= WB:
                        nc.gpsimd.tensor_tensor(
                            t[:, 0:128], t[:, 0:128], m_sb[:, 1, :], op=OP.add
                        )
                    st["t"] = t
                    st.pop("ps")

                # ---- stage C: exp (+row sums), recip, y for iter j-2 ----
                m = j - 2
                if 0 <= m < M_ITERS:
                    st = state[m]
                    e_full = epool.tile([128, 1152], bf16, tag="e")
                    e = e_full[:, :st["wc"]]
                    l_sb = spool.tile([128, 1], f32, tag="l")
                    nc.scalar.activation(
                        e, st["t"], AF.Exp, scale=CAP, bias=bcap, accum_out=l_sb
                    )
                    r_sb = spool.tile([128, 1], f32, tag="r")
                    nc.vector.reciprocal_approx_fast(r_sb, l_sb)
                    y_full = ypool.tile([128, 1152], f16, tag="y")
                    y = y_full[:, :st["wc"]]
                    yp = nc.vector.tensor_scalar(
                        y, e, r_sb, 0.03 / 1.06, op0=OP.mult, op1=OP.subtract
                    )
                    yp.ins.bass_priority = max(0, tc.cur_priority - 60)
                    st["y"] = y
                    st.pop("t")

                # ---- stage D: PE transpose + clamp copy for iter j-3 ----
                m = j - 3
                if 0 <= m < M_ITERS:
                    st = state[m]
                    nW = st["nW"]
                    a2_full = apool.tile([128, 1152], f16, tag="a2")
                    a2 = a2_full[:, :st["wc"]]
                    for qd in range((nW + 3) // 4):
                        nblk = min(4, nW - qd * 4)
                        quad = ptp.tile([128, 512], f16, tag="pt")
                        for wb in range(nblk):
                            w = qd * 4 + wb
                            tp = nc.tensor.transpose(
                                quad[:, wb * 128:(wb + 1) * 128],
                                st["y"][:, w * 128:(w + 1) * 128],
                                i_sb,
                            )
                            tp.ins.bass_priority = max(0, tc.cur_priority - 120)
                        cl = nc.vector.tensor_scalar(
                            a2[:, qd * 512: qd * 512 + nblk * 128],
                            quad[:, : nblk * 128],
                            0.0,
                            1.0 / 1.06,
                            op0=OP.max,
                            op1=OP.min,
                        )
                        cl.ins.bass_priority = max(0, tc.cur_priority - 120)
                    st["a2"] = a2
                    st.pop("y")

                # ---- stage E: AV + output DMA for iter j-4 ----
                m = j - 4
                if 0 <= m < M_ITERS:
                    st = state.pop(m)
                    h, i = it(m)
                    nW = st["nW"]
                    j0 = st["j0"]
                    po = pout.tile([128, 128], f32, tag="po")
                    for w in range(nW):
                        av = nc.tensor.matmul(
                            po,
                            lhsT=st["a2"][:, w * 128:(w + 1) * 128],
                            rhs=v_sb[:, j0 + w, :],
                            start=(w == 0),
                            stop=(w == nW - 1),
                        )
                        av.ins.bass_priority = max(0, tc.cur_priority - 60)
                    o_sb = opool.tile([128, 128], f16, tag="o")
                    nc.vector.tensor_copy(o_sb, po)
                    nc.sync.dma_start(out[i * 128:(i + 1) * 128, h, :], o_sb)

    nc.compile()
    return nc


def _host_inputs(q, k, v):
    q = np.asarray(q, dtype=np.float32)
    k = np.asarray(k, dtype=np.float32)
    v = np.asarray(v, dtype=np.float32)

    a = np.arange(128)
    mask_diag = np.where(a[None, :] <= a[:, None], 0.0, MASK_VAL).astype(np.float32)
    mask_left = np.where(a[None, :] >= a[:, None], 0.0, MASK_VAL).astype(np.float32)
    msk = np.stack([mask_diag, mask_left]).astype(np.float32)
    idn = np.eye(128, dtype=np.float16)

    in_maps = []
    for c in range(8):
        b = c // 4
        g = c % 4
        qTc = np.ascontiguousarray(
            q[b, :, 4 * g:4 * g + 4, :].transpose(1, 2, 0)
        ).astype(np.float16)
        kTh = np.ascontiguousarray(k[b, :, g, :].T).astype(np.float16)
        vhh = (np.float32(1.06) * np.ascontiguousarray(v[b, :, g, :])).astype(np.float16)
        in_maps.append({"qT": qTc, "kT": kTh, "vh": vhh, "msk": msk, "idn": idn})
    return in_maps


def _run(q, k, v, trace=False):
    from concourse.bass_utils import run_bass_kernel_spmd

    if "nc" not in _CACHED:
        _CACHED["nc"] = _build_bass()
    nc = _CACHED["nc"]

    in_maps = _host_inputs(q, k, v)
    res = run_bass_kernel_spmd(nc, in_maps, list(range(8)), trace=trace)

    out = np.zeros((B, S, HQ, D), np.float32)
    for c in range(8):
        b = c // 4
        g = c % 4
        out[b, :, 4 * g:4 * g + 4, :] = res.results[c]["out"]
    return out, res


def kernel(q, k, v):
    out, _ = _run(q, k, v, trace=False)
    return out


def kernel_traced(q, k, v):
    out, res = _run(q, k, v, trace=True)
    return out, res
